# revision 1
# baseline (speedup 1.0000x reference)
"""ConvLSTM classifier kernel for Trainium2 (8 NeuronCores, data-parallel).

Math (per core, batch shard BL):
  for t in 0..T-1:
    gates = conv1d(x_t, w_ih) + conv1d(h, w_hh) + bias     # (BL, 20, 64), 'SAME' K=5
    i,f,g,o = split(gates); i,f,o = sigmoid; g = tanh
    c = f*c + i*g ; h = o*tanh(c)
  logit = h . fc_w + fc_b ; p = sigmoid(logit)
  out = 1 - prod_c(1-p_c) * (1-sigmoid(baseline))

Implementation strategy:
  - batch on SBUF partitions everywhere (128-batch groups).
  - conv pair as ONE PE matmul per l-window of 8 outputs: stationary lhsT =
    im2col data tile [128 taps, 128 batch] (built by xbar DMA transpose),
    moving rhs = banded weight matrix W_band [128, 160] (shared by all
    windows; bias via a constant-1 row; g-block weights doubled so a single
    Sigmoid pass covers all four gates: tanh(g) = 2*sigmoid(2g)-1).
  - gates land in PSUM [128b, 160] fp32; ScalarE sigmoid reads multi-window
    bank-strided APs out of a 3-bank PSUM slot (2 rotating slots).
  - VectorE does the cell update with fused scalar_tensor_tensor ops,
    c kept in fp32 ping-pong buffers; GpSimd does h = o*tanh(c) and the
    h im2col window scatter for the next step's transpose source.
"""

import numpy as np

import concourse.bass as bass
import concourse.bacc as bacc
import concourse.tile as tile
import concourse.mybir as mybir
from concourse import bass_utils

dt = mybir.dt
ALU = mybir.AluOpType
ACT = mybir.ActivationFunctionType

TIME = 16
BATCH = 16384
C = 5
L = 64
NCORES = 8
BL = BATCH // NCORES          # 2048 per core
NW = 8                        # l-windows per row (l_seg = 8)
WJ = 12                       # taps per (window, channel): 8 + 4 halo

# xh_pre column layout per window w (128 cols each):
#   cols  0..59   x taps: c*12 + j  -> x[b, c, w*8 + j - 2]
#   col   124     constant 1.0 (bias row)
#   cols 64..123  h taps: 64 + c*12 + j -> h[b, c, w*8 + j - 2]
#   everything else stays zero (conv edge padding + unused W rows)
X_OFF = 0
H_OFF = 64
BIAS_COL = 124


def make_wband(w_ih, w_hh, b_ih, b_hh):
    """Banded weight matrix [128, 160] fp16.

    Rows match xh_pre columns (after transpose these are lhsT partitions).
    Cols: G*40 + ch*8 + lam, G in (i,f,o,g) order, lam = within-window l.
    g-block (G=3) is doubled for the tanh-via-sigmoid trick.
    """
    refbase = (0, 5, 15, 10)  # i, f, o, g -> reference channel offsets
    wb = np.zeros((128, 160), np.float32)
    for row0, wmat in ((X_OFF, w_ih), (H_OFF, w_hh)):
        for c in range(C):
            for j in range(WJ):
                r = row0 + c * WJ + j
                for G in range(4):
                    scale = 2.0 if G == 3 else 1.0
                    for ch in range(C):
                        for lam in range(NW):
                            k = j - lam
                            if 0 <= k < 5:
                                wb[r, G * 40 + ch * 8 + lam] = (
                                    scale * wmat[refbase[G] + ch, c, k]
                                )
    bias = (np.asarray(b_ih) + np.asarray(b_hh)).astype(np.float32)
    for G in range(4):
        scale = 2.0 if G == 3 else 1.0
        for ch in range(C):
            for lam in range(NW):
                wb[BIAS_COL, G * 40 + ch * 8 + lam] = scale * bias[refbase[G] + ch]
    return wb.astype(np.float16)


def _ap(base, off, dims):
    """Manual AP over the same tensor as `base` (an AP), keeping its
    partition dim, with free dims `dims` at extra element offset `off`."""
    return bass.AP(
        tensor=base.tensor,
        offset=base.offset + off,
        ap=[list(base.ap[0])] + [list(d) for d in dims],
    )


def build_body(tc, out_dram, xs, wband_d, fcw5_d, consts_d, T, nbg,
               no_xbar=False, stage=5):
    nc = tc.nc
    f16, f32 = dt.float16, dt.float32

    from contextlib import ExitStack
    es = ExitStack()
    pers = es.enter_context(tc.tile_pool(name="pers", bufs=1))
    psum_pool = es.enter_context(tc.tile_pool(name="psum", bufs=2, space="PSUM"))
    ifog_pool = es.enter_context(tc.tile_pool(name="ifog", bufs=6))
    small = es.enter_context(tc.tile_pool(name="small", bufs=6))
    xht_pool = es.enter_context(tc.tile_pool(name="xht", bufs=4))
    fin_pool = es.enter_context(tc.tile_pool(name="fin", bufs=2))

    wband = pers.tile([128, 160], f16, tag="wband")
    nc.sync.dma_start(out=wband, in_=wband_d)
    fcw5 = pers.tile([128, 5 * L], f16, tag="fcw5")
    nc.gpsimd.dma_start(
        out=fcw5,
        in_=bass.AP(tensor=fcw5_d.tensor, offset=fcw5_d.offset,
                    ap=[[0, 128], [1, 5 * L]]),
    )
    consts = pers.tile([128, 2], f32, tag="consts")
    nc.gpsimd.dma_start(
        out=consts,
        in_=bass.AP(tensor=consts_d.tensor, offset=consts_d.offset,
                    ap=[[0, 128], [1, 2]]),
    )
    fcbneg = consts[:, 0:1]
    negq = consts[:, 1:2]

    xh = []
    for bg in range(nbg):
        pair = []
        for pp in range(2):
            t_ = pers.tile([128, NW * 128], f16, tag=f"xh{bg}_{pp}",
                           name=f"xh{bg}_{pp}")
            nc.vector.memset(t_, 0.0)
            w3i = t_[:].rearrange("p (w r) -> p w r", r=128)
            nc.vector.memset(w3i[:, :, BIAS_COL : BIAS_COL + 1], 1.0)
            pair.append(t_)
        xh.append(pair)

    npair = (nbg + 1) // 2
    cbuf = [[pers.tile([128, 640], f16, tag=f"c{pp}_{pr}", name=f"c{pp}_{pr}")
             for pr in range(npair)] for pp in range(2)]
    for pr in range(npair):
        nc.vector.memset(cbuf[0][pr], 0.0)
    tpair = [pers.tile([128, 640], f16, tag=f"t{pr}", name=f"t{pr}")
             for pr in range(npair)]

    o_slices = {}
    for bg in range(nbg):
        nc.gpsimd.dma_start(
            out=_ap(xh[bg][0][:], X_OFF, [[128, NW], [1, C * WJ]]),
            in_=xs[0, bg * 128 : (bg + 1) * 128].rearrange(
                "b (w r) -> b w r", r=C * WJ),
        )
    for t in range(T):
        c_old, c_new = cbuf[t % 2], cbuf[(t + 1) % 2]
        for bg in range(nbg):
            xh_full = xh[bg][t % 2][:]
            xh_next = xh[bg][(t + 1) % 2][:]

            if stage < 2:
                continue
            if not no_xbar:
                xht = xht_pool.tile([128, NW, 128], f16, tag="xht")
                nc.sync.dma_start(out=xht[:], in_=xh_full, transpose=True)
                lhsTs = [xht[:, w, :] for w in range(NW)]
            else:
                w3x = xh_full.rearrange("p (w r) -> p w r", r=128)
                lhsTs = [w3x[:, w, :] for w in range(NW)]

            slot = psum_pool.tile([128, 4 * 512], f32, tag="gates")
            for w in range(NW):
                col = (w // 2) * 512 + (w % 2) * 160
                nc.tensor.matmul(
                    slot[:, col : col + 160],
                    lhsT=lhsTs[w],
                    rhs=wband[:],
                    start=True,
                    stop=True,
                )

            if stage < 3:
                continue
            ifog = ifog_pool.tile([128, NW * 160], f16, tag="ifog")
            sfull = slot[:]
            nc.scalar.activation(
                out=_ap(ifog[:], 0, [[320, 4], [160, 2], [1, 160]]),
                in_=_ap(sfull, 0, [[512, 4], [160, 2], [1, 160]]),
                func=ACT.Sigmoid,
            )

            if stage < 4:
                continue
            ifog_f = ifog[:]
            sl_i = _ap(ifog_f, 0, [[160, NW], [1, 40]])
            sl_f = _ap(ifog_f, 40, [[160, NW], [1, 40]])
            sl_g = _ap(ifog_f, 120, [[160, NW], [1, 40]])
            o_slices[bg] = _ap(ifog_f, 80, [[160, NW], [8, C], [1, 8]])

            v = small.tile([128, 320], f16, tag="v")
            nc.vector.tensor_tensor(out=v, in0=sl_i, in1=sl_g, op=ALU.mult)
            u = small.tile([128, 320], f16, tag="u")
            nc.vector.scalar_tensor_tensor(
                out=u, in0=v[:], scalar=2.0, in1=sl_i,
                op0=ALU.mult, op1=ALU.subtract,
            )
            co = c_old[bg // 2][:, (bg % 2) * 320 : (bg % 2 + 1) * 320]
            cn = c_new[bg // 2][:, (bg % 2) * 320 : (bg % 2 + 1) * 320]
            fc = small.tile([128, 320], f16, tag="fc")
            nc.vector.tensor_tensor(out=fc, in0=sl_f, in1=co, op=ALU.mult)
            nc.vector.tensor_tensor(out=cn, in0=fc[:], in1=u[:], op=ALU.add)

            if bg % 2 == 1 or bg == nbg - 1:
                blo = bg - 1 if bg % 2 == 1 else bg
                pr = bg // 2
                w_hi = (bg % 2 + 1) * 320
                nc.scalar.activation(
                    out=tpair[pr][:, 0:w_hi], in_=c_new[pr][:, 0:w_hi],
                    func=ACT.Tanh
                )
                for b2 in range(blo, bg + 1):
                    tsl = _ap(tpair[pr][:, (b2 % 2) * 320 : (b2 % 2 + 1) * 320], 0,
                              [[40, NW], [8, C], [1, 8]])
                    xh2 = xh[b2][(t + 1) % 2][:]
                    hdst = _ap(xh2, H_OFF + 2, [[128, NW], [WJ, C], [1, 8]])
                    nc.vector.tensor_tensor(
                        out=hdst, in0=o_slices[b2], in1=tsl, op=ALU.mult
                    )
                    nc.vector.tensor_copy(
                        out=_ap(xh2, 128 + H_OFF, [[128, NW - 1], [WJ, C], [1, 2]]),
                        in_=_ap(xh2, H_OFF + 8, [[128, NW - 1], [WJ, C], [1, 2]]),
                    )
                    nc.vector.tensor_copy(
                        out=_ap(xh2, H_OFF + 10, [[128, NW - 1], [WJ, C], [1, 2]]),
                        in_=_ap(xh2, 128 + H_OFF + 2, [[128, NW - 1], [WJ, C], [1, 2]]),
                    )

        if t + 1 < T:
            for bg in range(nbg):
                nc.gpsimd.dma_start(
                    out=_ap(xh[bg][(t + 1) % 2][:], X_OFF,
                            [[128, NW], [1, C * WJ]]),
                    in_=xs[t + 1, bg * 128 : (bg + 1) * 128].rearrange(
                        "b (w r) -> b w r", r=C * WJ),
                )

    # --- final FC / combine ---
    for bg in range(nbg):
        hview = _ap(xh[bg][T % 2][:], H_OFF + 2, [[128, NW], [WJ, C], [1, 8]])
        fview = _ap(fcw5[:], 0, [[8, NW], [L, C], [1, 8]])
        tmp5 = fin_pool.tile([128, C * L], f32, tag="tmp5")
        tview = _ap(tmp5[:], 0, [[8, NW], [L, C], [1, 8]])
        nc.vector.tensor_tensor(out=tview, in0=hview, in1=fview, op=ALU.mult)
        nraw = fin_pool.tile([128, C], f32, tag="nraw")
        nc.vector.tensor_reduce(
            out=nraw,
            in_=tmp5[:].rearrange("p (c l) -> p c l", l=L),
            axis=mybir.AxisListType.X,
            op=ALU.add,
        )
        pbar = fin_pool.tile([128, C], f32, tag="pbar")
        nc.scalar.activation(
            out=pbar, in_=nraw[:], func=ACT.Sigmoid, bias=fcbneg, scale=1.0
        )
        q2 = fin_pool.tile([128, 2], f32, tag="q2")
        nc.vector.tensor_tensor(out=q2, in0=pbar[:, 0:2], in1=pbar[:, 2:4],
                                op=ALU.mult)
        prod = fin_pool.tile([128, 1], f32, tag="prod")
        nc.vector.tensor_tensor(out=prod, in0=q2[:, 0:1], in1=q2[:, 1:2],
                                op=ALU.mult)
        nc.vector.tensor_tensor(out=prod, in0=prod[:], in1=pbar[:, 4:5],
                                op=ALU.mult)
        res = fin_pool.tile([128, 1], f32, tag="res")
        nc.scalar.activation(
            out=res, in_=prod[:], func=ACT.Identity, bias=1.0, scale=negq
        )
        nc.sync.dma_start(out=out_dram[bg], in_=res[:])
    es.close()


def window_x(x):
    """[T, B, 5, 64] fp32 -> [T, B, NW*60] fp16 im2col, col = w*60 + c*12 + j."""
    from numpy.lib.stride_tricks import sliding_window_view
    xp = np.pad(x, ((0, 0), (0, 0), (0, 0), (2, 2)))
    win = sliding_window_view(xp, WJ, axis=3)[:, :, :, ::NW, :]  # T,B,C,NW,WJ
    return np.ascontiguousarray(
        win.transpose(0, 1, 3, 2, 4), dtype=np.float16       # T,B,NW,C,WJ
    ).reshape(x.shape[0], x.shape[1], NW * C * WJ)


def host_prep(w_ih, w_hh, b_ih, b_hh, fc_w, fc_b, baseline):
    wband = make_wband(np.asarray(w_ih), np.asarray(w_hh),
                       np.asarray(b_ih), np.asarray(b_hh))
    fcw = np.asarray(fc_w)[0].astype(np.float32)          # (64,)
    fcw5 = np.tile(-fcw, C)[None, :].astype(np.float16)    # (1, 320)
    base = float(np.asarray(baseline)[0])
    sig_base = 1.0 / (1.0 + np.exp(-base))
    consts = np.array([[-float(np.asarray(fc_b)[0]), -(1.0 - sig_base)]],
                      np.float32)
    return wband, fcw5, consts


def build_program(T, nbg, no_xbar=False, stage=5):
    nc = bacc.Bacc("TRN2", target_bir_lowering=False, debug=False, num_devices=1)
    xs = nc.dram_tensor("xs", [T, nbg * 128, NW * C * WJ], dt.float16,
                        kind="ExternalInput").ap()
    wband_d = nc.dram_tensor("wband", [128, 160], dt.float16,
                             kind="ExternalInput").ap()
    fcw5_d = nc.dram_tensor("fcw5", [1, C * L], dt.float16,
                            kind="ExternalInput").ap()
    consts_d = nc.dram_tensor("consts", [1, 2], dt.float32,
                              kind="ExternalInput").ap()
    out_d = nc.dram_tensor("out", [nbg, 128], dt.float32,
                           kind="ExternalOutput").ap()
    with tile.TileContext(nc) as tc:
        build_body(tc, out_d, xs, wband_d, fcw5_d, consts_d, T, nbg,
                   no_xbar=no_xbar, stage=stage)
    nc.compile()
    return nc


_PROG_CACHE = {}


def kernel(x, w_ih, w_hh, b_ih, b_hh, fc_w, fc_b, baseline):
    x = np.asarray(x)
    T, B = x.shape[0], x.shape[1]
    nbg = (B // NCORES) // 128
    key = (T, nbg)
    if key not in _PROG_CACHE:
        _PROG_CACHE[key] = build_program(T, nbg)
    nc = _PROG_CACHE[key]

    wband, fcw5, consts = host_prep(w_ih, w_hh, b_ih, b_hh, fc_w, fc_b, baseline)
    xw = window_x(x)
    bl = B // NCORES
    in_maps = []
    for core in range(NCORES):
        in_maps.append({
            "xs": np.ascontiguousarray(xw[:, core * bl : (core + 1) * bl]),
            "wband": wband,
            "fcw5": fcw5,
            "consts": consts,
        })
    res = bass_utils.run_bass_kernel_spmd(nc, in_maps, core_ids=list(range(NCORES)))
    out = np.concatenate([r["out"].reshape(-1) for r in res.results])
    return out.astype(np.float32)



# revision 3
# speedup vs baseline: 1.8176x; 1.8176x over previous
"""ConvLSTM classifier kernel for Trainium2 (8 NeuronCores, data-parallel).

Math (per core, batch shard BL=2048):
  for t in 0..T-1:
    gates = conv1d(x_t, w_ih) + conv1d(h, w_hh) + bias     # (BL, 20, 64), 'SAME' K=5
    i,f,o = sigmoid; g = tanh
    c = f*c + i*g ; h = o*tanh(c)
  logit = h . fc_w + fc_b ; p = sigmoid(logit)
  out = 1 - prod_c(1-p_c) * (1-sigmoid(baseline))

Implementation (per 128-batch group "block", 16 blocks x 16 steps):
  - batch on SBUF partitions everywhere.
  - x is im2col'd AND padded on host into full 128-col window blocks
    [T, B, 8, 128]: cols 0..59 x-taps, col 124 = 1.0 (bias row), rest 0.
    One contiguous 4MB DMA per step loads a whole-step mega-tile
    [128, 16bg x 8w x 128]; triple-buffered so the load never collides
    with the in-flight transposes (Tile serializes xbar transposes
    against other DMA completions).
  - per block: one xbar DMA transpose [128b, 1024] -> [128 taps, 8, 128b]
    builds the matmul lhsT (x-taps + h-taps + bias in one 128-row tile,
    single FWL LDWEIGHTS per window).
  - 8 matmuls (one per 8-wide l-window) vs banded weights [128, 160].
  - ScalarE: one sigmoid over i,f,o (960), tanh over g (320), and a
    deferred tanh(c) for the previous block (320, reordered to
    channel-major h layout with zero-guard halo columns).
  - VectorE: v=i*g, fc=f*c, c=fc+v (contiguous, 2x mode) and
    h = o*tanh(c) into the guarded channel-major h tile.
  - GpSimd: window-expands h (12 taps incl halos, via the guard cols)
    into next step's mega-tile h-region.
"""

import numpy as np

import concourse.bass as bass
import concourse.bacc as bacc
import concourse.tile as tile
import concourse.mybir as mybir
from concourse import bass_utils

dt = mybir.dt
ALU = mybir.AluOpType
ACT = mybir.ActivationFunctionType

TIME = 16
BATCH = 16384
C = 5
L = 64
NCORES = 8
BL = BATCH // NCORES          # 2048 per core
NW = 8                        # l-windows per row (l_seg = 8)
WJ = 12                       # taps per (window, channel): 8 + 4 halo
X_OFF = 0
H_OFF = 64
BIAS_COL = 124
CG = 68                       # channel block in clmaj layout: 2+64+2 guards


def make_wband(w_ih, w_hh, b_ih, b_hh):
    """Banded weight matrix [128, 160] fp16.

    Rows match window-block columns (after transpose these are lhsT
    partitions). Cols: G*40 + ch*8 + lam, G in (i,f,o,g) order.
    """
    refbase = (0, 5, 15, 10)  # i, f, o, g -> reference channel offsets
    wb = np.zeros((128, 160), np.float32)
    for row0, wmat in ((X_OFF, w_ih), (H_OFF, w_hh)):
        for c in range(C):
            for j in range(WJ):
                r = row0 + c * WJ + j
                for G in range(4):
                    for ch in range(C):
                        for lam in range(NW):
                            k = j - lam
                            if 0 <= k < 5:
                                wb[r, G * 40 + ch * 8 + lam] = (
                                    wmat[refbase[G] + ch, c, k]
                                )
    bias = (np.asarray(b_ih) + np.asarray(b_hh)).astype(np.float32)
    for G in range(4):
        for ch in range(C):
            for lam in range(NW):
                wb[BIAS_COL, G * 40 + ch * 8 + lam] = bias[refbase[G] + ch]
    return wb.astype(np.float16)


def _ap(base, off, dims):
    """Manual AP over the same tensor as `base` (an AP), keeping its
    partition dim, with free dims `dims` at extra element offset `off`."""
    return bass.AP(
        tensor=base.tensor,
        offset=base.offset + off,
        ap=[list(base.ap[0])] + [list(d) for d in dims],
    )


def build_body(tc, out_dram, xs, wband_d, fcw5_d, consts_d, T, nbg):
    nc = tc.nc
    f16, f32 = dt.float16, dt.float32

    from contextlib import ExitStack
    es = ExitStack()
    pers = es.enter_context(tc.tile_pool(name="pers", bufs=1))
    psum_pool = es.enter_context(tc.tile_pool(name="psum", bufs=2, space="PSUM"))
    xht_pool = es.enter_context(tc.tile_pool(name="xht", bufs=8))
    ifo_pool = es.enter_context(tc.tile_pool(name="ifo", bufs=4))
    g_pool = es.enter_context(tc.tile_pool(name="g", bufs=4))
    vfc_pool = es.enter_context(tc.tile_pool(name="vfc", bufs=4))
    tcl_pool = es.enter_context(tc.tile_pool(name="tcl", bufs=4))
    fin_pool = es.enter_context(tc.tile_pool(name="fin", bufs=4))

    wband = pers.tile([128, 160], f16, tag="wband")
    nc.sync.dma_start(out=wband, in_=wband_d)
    fcw5 = pers.tile([128, C * L], f16, tag="fcw5")
    nc.gpsimd.dma_start(
        out=fcw5,
        in_=bass.AP(tensor=fcw5_d.tensor, offset=fcw5_d.offset,
                    ap=[[0, 128], [1, C * L]]),
    )
    consts = pers.tile([128, 2], f32, tag="consts")
    nc.gpsimd.dma_start(
        out=consts,
        in_=bass.AP(tensor=consts_d.tensor, offset=consts_d.offset,
                    ap=[[0, 128], [1, 2]]),
    )
    fcbneg = consts[:, 0:1]
    negq = consts[:, 1:2]

    # step mega-tiles: [128, nbg * NW * 128] fp16, triple-buffered over t%3
    NXB = 3
    colsz = nbg * NW * 128
    xh = [pers.tile([128, colsz], f16, tag=f"xh{k}", name=f"xh{k}")
          for k in range(NXB)]

    # channel-major h tiles with zero-guard halo cols, one per block (pers)
    h_cl = pers.tile([128, nbg * C * CG], f16, tag="h_cl")
    nc.vector.memset(h_cl, 0.0)

    # c state: ping-pong per block
    cbuf = [[pers.tile([128, 320], f16, tag=f"c{pp}_{bg}", name=f"c{pp}_{bg}")
             for bg in range(nbg)] for pp in range(2)]
    for bg in range(nbg):
        nc.vector.memset(cbuf[0][bg], 0.0)

    out_acc = pers.tile([128, nbg], f32, tag="out_acc")

    def load_step(t):
        src = xs[t].rearrange("(g p) c -> p g c", p=128)
        nc.gpsimd.dma_start(out=xh[t % NXB][:], in_=src)

    # prologue: steps 0 and 1
    load_step(0)
    load_step(1)

    ifo_tiles = {}
    cn_tiles = {}
    pending = None

    def tail(bg, t):
        """Deferred: tanh(c), h = o*tanh(c) (clmaj), window-expand into
        next step's mega-tile h-region."""
        ifo_f = ifo_tiles.pop(bg)[:]
        cn = cn_tiles.pop(bg)
        tanh_cl = tcl_pool.tile([128, C * CG], f16, tag="tcl")
        # (w, ch, lam) contiguous -> clmaj col = 2 + ch*68 + (8w+lam)
        nc.scalar.activation(
            out=_ap(tanh_cl[:], 2, [[8, NW], [CG, C], [1, 8]]),
            in_=_ap(cn[:], 0, [[40, NW], [8, C], [1, 8]]),
            func=ACT.Tanh,
        )
        hsl = _ap(h_cl[:], bg * C * CG + 2, [[CG, C], [8, NW], [1, 8]])
        osl = _ap(ifo_f, 640, [[8, C], [40, NW], [1, 8]])
        tsl = _ap(tanh_cl[:], 2, [[CG, C], [8, NW], [1, 8]])
        nc.vector.tensor_tensor(out=hsl, in0=osl, in1=tsl, op=ALU.mult)
        if t + 1 < T:
            # expand h (with halos via guard cols) into xh[t+1] h-region
            dst = _ap(xh[(t + 1) % NXB][:], bg * NW * 128 + H_OFF,
                      [[128, NW], [WJ, C], [1, WJ]])
            srcap = _ap(h_cl[:], bg * C * CG, [[8, NW], [CG, C], [1, WJ]])
            nc.gpsimd.tensor_copy(out=dst, in_=srcap)

    for t in range(T):
        for bg in range(nbg):
            xht = xht_pool.tile([128, NW, 128], f16, tag="xht")
            nc.sync.dma_start(
                out=xht[:],
                in_=xh[t % NXB][:, bg * NW * 128:(bg + 1) * NW * 128],
                transpose=True,
            )

            slot = psum_pool.tile([128, 4 * 512], f32, tag="gates")
            for w in range(NW):
                col = (w // 2) * 512 + (w % 2) * 160
                nc.tensor.matmul(
                    slot[:, col:col + 160],
                    lhsT=xht[:, w, :],
                    rhs=wband[:],
                    start=True,
                    stop=True,
                )

            sfull = slot[:]
            ifo = ifo_pool.tile([128, 960], f16, tag="ifo")
            # i,f,o: PSUM (bp, wip, G*40+chlam 0..119) -> gate-major blocks
            nc.scalar.activation(
                out=_ap(ifo[:], 0, [[80, 4], [40, 2], [320, 3], [1, 40]]),
                in_=_ap(sfull, 0, [[512, 4], [160, 2], [1, 120]]),
                func=ACT.Sigmoid,
            )
            g = g_pool.tile([128, 320], f16, tag="g")
            nc.scalar.activation(
                out=_ap(g[:], 0, [[80, 4], [40, 2], [1, 40]]),
                in_=_ap(sfull, 120, [[512, 4], [160, 2], [1, 40]]),
                func=ACT.Tanh,
            )

            ifo_f = ifo[:]
            v = vfc_pool.tile([128, 320], f16, tag="v")
            nc.vector.tensor_tensor(out=v, in0=ifo_f[:, 0:320], in1=g[:],
                                    op=ALU.mult)
            fc = vfc_pool.tile([128, 320], f16, tag="fc")
            co = cbuf[t % 2][bg]
            nc.vector.tensor_tensor(out=fc, in0=ifo_f[:, 320:640], in1=co[:],
                                    op=ALU.mult)
            cn = cbuf[(t + 1) % 2][bg]
            nc.vector.tensor_tensor(out=cn[:], in0=v[:], in1=fc[:], op=ALU.add)

            ifo_tiles[bg] = ifo
            cn_tiles[bg] = cn

            if pending is not None:
                tail(*pending)
            pending = (bg, t)

        if t + 2 < T:
            load_step(t + 2)

    tail(*pending)

    # --- final FC / combine ---
    for bg in range(nbg):
        hsl = _ap(h_cl[:], bg * C * CG + 2, [[CG, C], [1, L]])
        fview = _ap(fcw5[:], 0, [[L, C], [1, L]])
        tmp5 = fin_pool.tile([128, C * L], f32, tag="tmp5")
        nc.vector.tensor_tensor(
            out=_ap(tmp5[:], 0, [[L, C], [1, L]]),
            in0=hsl, in1=fview, op=ALU.mult)
        nraw = fin_pool.tile([128, C], f32, tag="nraw")
        nc.vector.tensor_reduce(
            out=nraw,
            in_=tmp5[:].rearrange("p (c l) -> p c l", l=L),
            axis=mybir.AxisListType.X,
            op=ALU.add,
        )
        pbar = fin_pool.tile([128, C], f32, tag="pbar")
        nc.scalar.activation(
            out=pbar, in_=nraw[:], func=ACT.Sigmoid, bias=fcbneg, scale=1.0
        )
        q2 = fin_pool.tile([128, 2], f32, tag="q2")
        nc.vector.tensor_tensor(out=q2, in0=pbar[:, 0:2], in1=pbar[:, 2:4],
                                op=ALU.mult)
        prod = fin_pool.tile([128, 1], f32, tag="prod")
        nc.vector.tensor_tensor(out=prod, in0=q2[:, 0:1], in1=q2[:, 1:2],
                                op=ALU.mult)
        nc.vector.tensor_tensor(out=prod, in0=prod[:], in1=pbar[:, 4:5],
                                op=ALU.mult)
        nc.scalar.activation(
            out=out_acc[:, bg:bg + 1], in_=prod[:], func=ACT.Identity,
            bias=1.0, scale=negq
        )
    nc.gpsimd.dma_start(out=out_dram, in_=out_acc[:])
    es.close()


def window_x(x):
    """[T, B, 5, 64] fp32 -> [T, B, NW*128] fp16 padded window blocks.

    Block w cols: 0..59 = x[b, c, w*8 + j - 2] (c*12 + j), col 124 = 1.0,
    everything else 0.
    """
    from numpy.lib.stride_tricks import sliding_window_view
    T, B = x.shape[0], x.shape[1]
    xp = np.pad(x, ((0, 0), (0, 0), (0, 0), (2, 2)))
    win = sliding_window_view(xp, WJ, axis=3)[:, :, :, ::NW, :]  # T,B,C,NW,WJ
    out = np.zeros((T, B, NW, 128), np.float16)
    out[:, :, :, : C * WJ] = (
        win.transpose(0, 1, 3, 2, 4).reshape(T, B, NW, C * WJ)
    )
    out[:, :, :, BIAS_COL] = 1.0
    return out.reshape(T, B, NW * 128)


def host_prep(w_ih, w_hh, b_ih, b_hh, fc_w, fc_b, baseline):
    wband = make_wband(np.asarray(w_ih), np.asarray(w_hh),
                       np.asarray(b_ih), np.asarray(b_hh))
    fcw = np.asarray(fc_w)[0].astype(np.float32)          # (64,)
    fcw5 = np.tile(-fcw, C)[None, :].astype(np.float16)    # (1, 320)
    base = float(np.asarray(baseline)[0])
    sig_base = 1.0 / (1.0 + np.exp(-base))
    consts = np.array([[-float(np.asarray(fc_b)[0]), -(1.0 - sig_base)]],
                      np.float32)
    return wband, fcw5, consts


def build_program(T, nbg):
    nc = bacc.Bacc("TRN2", target_bir_lowering=False, debug=False, num_devices=1)
    xs = nc.dram_tensor("xs", [T, nbg * 128, NW * 128], dt.float16,
                        kind="ExternalInput").ap()
    wband_d = nc.dram_tensor("wband", [128, 160], dt.float16,
                             kind="ExternalInput").ap()
    fcw5_d = nc.dram_tensor("fcw5", [1, C * L], dt.float16,
                            kind="ExternalInput").ap()
    consts_d = nc.dram_tensor("consts", [1, 2], dt.float32,
                              kind="ExternalInput").ap()
    out_d = nc.dram_tensor("out", [128, nbg], dt.float32,
                           kind="ExternalOutput").ap()
    with tile.TileContext(nc) as tc:
        build_body(tc, out_d, xs, wband_d, fcw5_d, consts_d, T, nbg)
    nc.compile()
    return nc


_PROG_CACHE = {}


def kernel(x, w_ih, w_hh, b_ih, b_hh, fc_w, fc_b, baseline):
    x = np.asarray(x)
    T, B = x.shape[0], x.shape[1]
    nbg = (B // NCORES) // 128
    key = (T, nbg)
    if key not in _PROG_CACHE:
        _PROG_CACHE[key] = build_program(T, nbg)
    nc = _PROG_CACHE[key]

    wband, fcw5, consts = host_prep(w_ih, w_hh, b_ih, b_hh, fc_w, fc_b, baseline)
    xw = window_x(x)
    bl = B // NCORES
    in_maps = []
    for core in range(NCORES):
        in_maps.append({
            "xs": np.ascontiguousarray(xw[:, core * bl: (core + 1) * bl]),
            "wband": wband,
            "fcw5": fcw5,
            "consts": consts,
        })
    res = bass_utils.run_bass_kernel_spmd(nc, in_maps, core_ids=list(range(NCORES)))
    out = np.concatenate([r["out"].T.reshape(-1) for r in res.results])
    return out.astype(np.float32)


# revision 10
# speedup vs baseline: 2.5171x; 1.3848x over previous
"""ConvLSTM classifier kernel for Trainium2 (8 NeuronCores, data-parallel).

Math (per core, batch shard BL=2048):
  for t in 0..T-1:
    gates = conv1d(x_t, w_ih) + conv1d(h, w_hh) + bias     # (BL, 20, 64), 'SAME' K=5
    i,f,o = sigmoid; g = tanh
    c = f*c + i*g ; h = o*tanh(c)
  logit = h . fc_w + fc_b ; p = sigmoid(logit)
  out = 1 - prod_c(1-p_c) * (1-sigmoid(baseline))

Implementation (per 128-batch group "block", 16 blocks x 16 steps):
  - batch on SBUF partitions everywhere.
  - x is im2col'd AND padded on host into full 128-col window blocks
    [T, B, 8, 128]: cols 0..59 x-taps, col 124 = 1.0 (bias row), rest 0.
    One contiguous 4MB DMA per step loads a whole-step mega-tile
    [128, 16bg x 8w x 128]; triple-buffered so the load never collides
    with the in-flight transposes (Tile serializes xbar transposes
    against other DMA completions).
  - per block: one xbar DMA transpose [128b, 1024] -> [128 taps, 8, 128b]
    builds the matmul lhsT (x-taps + h-taps + bias in one 128-row tile,
    single FWL LDWEIGHTS per window).
  - 8 matmuls (one per 8-wide l-window) vs banded weights [128, 160].
  - ScalarE: one sigmoid over i,f,o (960), tanh over g (320), and a
    deferred tanh(c) for the previous block (320, reordered to
    channel-major h layout with zero-guard halo columns).
  - VectorE: v=i*g, fc=f*c, c=fc+v (contiguous, 2x mode) and
    h = o*tanh(c) into the guarded channel-major h tile.
  - GpSimd: window-expands h (12 taps incl halos, via the guard cols)
    into next step's mega-tile h-region.
"""

import numpy as np

import concourse.bass as bass
import concourse.bacc as bacc
import concourse.tile as tile
import concourse.mybir as mybir
from concourse import bass_utils

dt = mybir.dt
ALU = mybir.AluOpType
ACT = mybir.ActivationFunctionType

TIME = 16
BATCH = 16384
C = 5
L = 64
NCORES = 8
BL = BATCH // NCORES          # 2048 per core
NW = 8                        # l-windows per row (l_seg = 8)
WJ = 12                       # taps per (window, channel): 8 + 4 halo
X_OFF = 0
H_OFF = 64
BIAS_COL = 124
CG = 68                       # channel block in clmaj layout: 2+64+2 guards


def make_wband(w_ih, w_hh, b_ih, b_hh):
    """Banded weight matrix [128, 160] fp16.

    Rows match window-block columns (after transpose these are lhsT
    partitions). Cols: G*40 + ch*8 + lam, G in (i,f,o,g) order.
    """
    refbase = (0, 5, 15, 10)  # i, f, o, g -> reference channel offsets
    wb = np.zeros((128, 160), np.float32)
    for row0, wmat in ((X_OFF, w_ih), (H_OFF, w_hh)):
        for c in range(C):
            for j in range(WJ):
                r = row0 + c * WJ + j
                for G in range(4):
                    scale = 2.0 if G == 3 else 1.0
                    for ch in range(C):
                        for lam in range(NW):
                            k = j - lam
                            if 0 <= k < 5:
                                wb[r, G * 40 + ch * 8 + lam] = (
                                    scale * wmat[refbase[G] + ch, c, k]
                                )
    bias = (np.asarray(b_ih) + np.asarray(b_hh)).astype(np.float32)
    for G in range(4):
        scale = 2.0 if G == 3 else 1.0
        for ch in range(C):
            for lam in range(NW):
                wb[BIAS_COL, G * 40 + ch * 8 + lam] = scale * bias[refbase[G] + ch]
    return wb.astype(np.float16)


def _ap(base, off, dims):
    """Manual AP over the same tensor as `base` (an AP), keeping its
    partition dim, with free dims `dims` at extra element offset `off`."""
    return bass.AP(
        tensor=base.tensor,
        offset=base.offset + off,
        ap=[list(base.ap[0])] + [list(d) for d in dims],
    )


def build_body(tc, out_dram, xs, wband_d, fcw5_d, consts_d, T, nbg):
    nc = tc.nc
    f16, f32 = dt.float16, dt.float32

    from contextlib import ExitStack
    es = ExitStack()
    pers = es.enter_context(tc.tile_pool(name="pers", bufs=1))
    psum_pool = es.enter_context(tc.tile_pool(name="psum", bufs=2, space="PSUM"))
    xht_pool = es.enter_context(tc.tile_pool(name="xht", bufs=8))
    ifog_pool = es.enter_context(tc.tile_pool(name="ifog", bufs=4))
    vfc_pool = es.enter_context(tc.tile_pool(name="vfc", bufs=4))
    tc_pool = es.enter_context(tc.tile_pool(name="tc", bufs=4))
    fin_pool = es.enter_context(tc.tile_pool(name="fin", bufs=4))

    wband = pers.tile([128, 160], f16, tag="wband")
    nc.sync.dma_start(out=wband, in_=wband_d)
    fcw5 = pers.tile([128, C * L], f16, tag="fcw5")
    nc.gpsimd.dma_start(
        out=fcw5,
        in_=bass.AP(tensor=fcw5_d.tensor, offset=fcw5_d.offset,
                    ap=[[0, 128], [1, C * L]]),
    )
    consts = pers.tile([128, 2], f32, tag="consts")
    nc.gpsimd.dma_start(
        out=consts,
        in_=bass.AP(tensor=consts_d.tensor, offset=consts_d.offset,
                    ap=[[0, 128], [1, 2]]),
    )
    fcbneg = consts[:, 0:1]
    negq = consts[:, 1:2]

    # step mega-tiles: [128, nbg * NW * 128] fp16, quad-buffered over t%4
    # (the h-tail TTs write into xh[(t+1)%4], whose zero-filled load must
    # complete a sweep earlier so the DVE never waits on it)
    NXB = 4
    colsz = nbg * NW * 128
    xh = [pers.tile([128, colsz], f16, tag=f"xh{k}", name=f"xh{k}")
          for k in range(NXB)]

    # c state: ping-pong per block
    cbuf = [[pers.tile([128, 320], f16, tag=f"c{pp}_{bg}", name=f"c{pp}_{bg}")
             for bg in range(nbg)] for pp in range(2)]
    for bg in range(nbg):
        nc.vector.memset(cbuf[0][bg], 0.0)

    out_acc = pers.tile([128, nbg], f32, tag="out_acc")

    def load_step(t):
        src = xs[t].rearrange("(g p) c -> p g c", p=128)
        nc.gpsimd.dma_start(out=xh[t % NXB][:], in_=src)

    # prologue: steps 0..2
    load_step(0)
    load_step(1)
    load_step(2)

    ifog_tiles = {}
    cn_tiles = {}
    pending = None

    def tail(bg, t):
        """Deferred: tanh(c), then h = o*tanh(c) window-expanded straight
        into the next step's mega-tile h-region (3 TTs: 8 core taps +
        left/right 2-tap halos from the neighbor windows; edge taps stay
        zero from the host-zeroed DMA load)."""
        ifog_f = ifog_tiles.pop(bg)[:]
        cn = cn_tiles.pop(bg)
        tch = tc_pool.tile([128, 320], f16, tag="tch")
        nc.scalar.activation(out=tch, in_=cn[:], func=ACT.Tanh)
        base = bg * NW * 128 + H_OFF
        dst = xh[(t + 1) % NXB][:]
        nc.vector.tensor_tensor(
            out=_ap(dst, base + 2, [[128, NW], [WJ, C], [1, 8]]),
            in0=_ap(ifog_f, 80, [[160, NW], [8, C], [1, 8]]),
            in1=_ap(tch[:], 0, [[40, NW], [8, C], [1, 8]]),
            op=ALU.mult,
        )
        nc.vector.tensor_tensor(
            out=_ap(dst, base + 128, [[128, NW - 1], [WJ, C], [1, 2]]),
            in0=_ap(ifog_f, 80 + 6, [[160, NW - 1], [8, C], [1, 2]]),
            in1=_ap(tch[:], 6, [[40, NW - 1], [8, C], [1, 2]]),
            op=ALU.mult,
        )
        nc.vector.tensor_tensor(
            out=_ap(dst, base + 10, [[128, NW - 1], [WJ, C], [1, 2]]),
            in0=_ap(ifog_f, 80 + 160, [[160, NW - 1], [8, C], [1, 2]]),
            in1=_ap(tch[:], 40, [[40, NW - 1], [8, C], [1, 2]]),
            op=ALU.mult,
        )

    for t in range(T):
        for bg in range(nbg):
            xht = xht_pool.tile([128, NW, 128], f16, tag="xht")
            nc.sync.dma_start(
                out=xht[:],
                in_=xh[t % NXB][:, bg * NW * 128:(bg + 1) * NW * 128],
                transpose=True,
            )

            slot = psum_pool.tile([128, 4 * 512], f32, tag="gates")
            for w in range(NW):
                col = (w // 2) * 512 + (w % 2) * 160
                nc.tensor.matmul(
                    slot[:, col:col + 160],
                    lhsT=xht[:, w, :],
                    rhs=wband[:],
                    start=True,
                    stop=True,
                )

            sfull = slot[:]
            ifog = ifog_pool.tile([128, NW * 160], f16, tag="ifog")
            nc.scalar.activation(
                out=_ap(ifog[:], 0, [[320, 4], [160, 2], [1, 160]]),
                in_=_ap(sfull, 0, [[512, 4], [160, 2], [1, 160]]),
                func=ACT.Sigmoid,
            )

            ifog_f = ifog[:]
            sl_i = _ap(ifog_f, 0, [[160, NW], [1, 40]])
            sl_f = _ap(ifog_f, 40, [[160, NW], [1, 40]])
            sl_g = _ap(ifog_f, 120, [[160, NW], [1, 40]])

            v = vfc_pool.tile([128, 320], f16, tag="v")
            nc.vector.tensor_tensor(out=v, in0=sl_i, in1=sl_g, op=ALU.mult)
            u = vfc_pool.tile([128, 320], f16, tag="u")
            nc.vector.scalar_tensor_tensor(
                out=u, in0=v[:], scalar=2.0, in1=sl_i,
                op0=ALU.mult, op1=ALU.subtract,
            )
            fc = vfc_pool.tile([128, 320], f16, tag="fc")
            co = cbuf[t % 2][bg]
            nc.vector.tensor_tensor(out=fc, in0=sl_f, in1=co[:], op=ALU.mult)
            cn = cbuf[(t + 1) % 2][bg]
            nc.vector.tensor_tensor(out=cn[:], in0=u[:], in1=fc[:], op=ALU.add)

            ifog_tiles[bg] = ifog
            cn_tiles[bg] = cn

            if pending is not None:
                tail(*pending)
            pending = (bg, t)

        if t + 3 < T:
            load_step(t + 3)

    tail(*pending)

    # --- final FC / combine ---
    hfin = xh[T % NXB][:]
    for bg in range(nbg):
        hview = _ap(hfin, bg * NW * 128 + H_OFF + 2, [[128, NW], [WJ, C], [1, 8]])
        fview = _ap(fcw5[:], 0, [[8, NW], [L, C], [1, 8]])
        tmp5 = fin_pool.tile([128, C * L], f32, tag="tmp5")
        nc.vector.tensor_tensor(
            out=_ap(tmp5[:], 0, [[8, NW], [L, C], [1, 8]]),
            in0=hview, in1=fview, op=ALU.mult)
        nraw = fin_pool.tile([128, C], f32, tag="nraw")
        nc.vector.tensor_reduce(
            out=nraw,
            in_=tmp5[:].rearrange("p (c l) -> p c l", l=L),
            axis=mybir.AxisListType.X,
            op=ALU.add,
        )
        pbar = fin_pool.tile([128, C], f32, tag="pbar")
        nc.scalar.activation(
            out=pbar, in_=nraw[:], func=ACT.Sigmoid, bias=fcbneg, scale=1.0
        )
        q2 = fin_pool.tile([128, 2], f32, tag="q2")
        nc.vector.tensor_tensor(out=q2, in0=pbar[:, 0:2], in1=pbar[:, 2:4],
                                op=ALU.mult)
        prod = fin_pool.tile([128, 1], f32, tag="prod")
        nc.vector.tensor_tensor(out=prod, in0=q2[:, 0:1], in1=q2[:, 1:2],
                                op=ALU.mult)
        nc.vector.tensor_tensor(out=prod, in0=prod[:], in1=pbar[:, 4:5],
                                op=ALU.mult)
        nc.scalar.activation(
            out=out_acc[:, bg:bg + 1], in_=prod[:], func=ACT.Identity,
            bias=1.0, scale=negq
        )
    nc.gpsimd.dma_start(out=out_dram, in_=out_acc[:])
    es.close()


def window_x(x):
    """[T, B, 5, 64] fp32 -> [T, B, NW*128] fp16 padded window blocks.

    Block w cols: 0..59 = x[b, c, w*8 + j - 2] (c*12 + j), col 124 = 1.0,
    everything else 0.
    """
    from numpy.lib.stride_tricks import sliding_window_view
    T, B = x.shape[0], x.shape[1]
    xp = np.pad(x, ((0, 0), (0, 0), (0, 0), (2, 2)))
    win = sliding_window_view(xp, WJ, axis=3)[:, :, :, ::NW, :]  # T,B,C,NW,WJ
    out = np.zeros((T, B, NW, 128), np.float16)
    out[:, :, :, : C * WJ] = (
        win.transpose(0, 1, 3, 2, 4).reshape(T, B, NW, C * WJ)
    )
    out[:, :, :, BIAS_COL] = 1.0
    return out.reshape(T, B, NW * 128)


def host_prep(w_ih, w_hh, b_ih, b_hh, fc_w, fc_b, baseline):
    wband = make_wband(np.asarray(w_ih), np.asarray(w_hh),
                       np.asarray(b_ih), np.asarray(b_hh))
    fcw = np.asarray(fc_w)[0].astype(np.float32)          # (64,)
    fcw5 = np.tile(-fcw, C)[None, :].astype(np.float16)    # (1, 320)
    base = float(np.asarray(baseline)[0])
    sig_base = 1.0 / (1.0 + np.exp(-base))
    consts = np.array([[-float(np.asarray(fc_b)[0]), -(1.0 - sig_base)]],
                      np.float32)
    return wband, fcw5, consts


def build_program(T, nbg):
    nc = bacc.Bacc("TRN2", target_bir_lowering=False, debug=False, num_devices=1)
    xs = nc.dram_tensor("xs", [T, nbg * 128, NW * 128], dt.float16,
                        kind="ExternalInput").ap()
    wband_d = nc.dram_tensor("wband", [128, 160], dt.float16,
                             kind="ExternalInput").ap()
    fcw5_d = nc.dram_tensor("fcw5", [1, C * L], dt.float16,
                            kind="ExternalInput").ap()
    consts_d = nc.dram_tensor("consts", [1, 2], dt.float32,
                              kind="ExternalInput").ap()
    out_d = nc.dram_tensor("out", [128, nbg], dt.float32,
                           kind="ExternalOutput").ap()
    with tile.TileContext(nc) as tc:
        build_body(tc, out_d, xs, wband_d, fcw5_d, consts_d, T, nbg)
    nc.compile()
    return nc


_PROG_CACHE = {}


def kernel(x, w_ih, w_hh, b_ih, b_hh, fc_w, fc_b, baseline):
    x = np.asarray(x)
    T, B = x.shape[0], x.shape[1]
    nbg = (B // NCORES) // 128
    key = (T, nbg)
    if key not in _PROG_CACHE:
        _PROG_CACHE[key] = build_program(T, nbg)
    nc = _PROG_CACHE[key]

    wband, fcw5, consts = host_prep(w_ih, w_hh, b_ih, b_hh, fc_w, fc_b, baseline)
    xw = window_x(x)
    bl = B // NCORES
    in_maps = []
    for core in range(NCORES):
        in_maps.append({
            "xs": np.ascontiguousarray(xw[:, core * bl: (core + 1) * bl]),
            "wband": wband,
            "fcw5": fcw5,
            "consts": consts,
        })
    res = bass_utils.run_bass_kernel_spmd(nc, in_maps, core_ids=list(range(NCORES)))
    out = np.concatenate([r["out"].T.reshape(-1) for r in res.results])
    return out.astype(np.float32)


# revision 15
# speedup vs baseline: 2.5234x; 1.0025x over previous
"""ConvLSTM classifier kernel for Trainium2 (8 NeuronCores, data-parallel).

Math (per core, batch shard BL=2048):
  for t in 0..T-1:
    gates = conv1d(x_t, w_ih) + conv1d(h, w_hh) + bias     # (BL, 20, 64), 'SAME' K=5
    i,f,o = sigmoid; g = tanh
    c = f*c + i*g ; h = o*tanh(c)
  logit = h . fc_w + fc_b ; p = sigmoid(logit)
  out = 1 - prod_c(1-p_c) * (1-sigmoid(baseline))

Implementation (per 128-batch group "block", 16 blocks x 16 steps):
  - batch on SBUF partitions everywhere.
  - x is im2col'd AND padded on host into full 128-col window blocks
    [T, B, 8, 128]: cols 0..59 x-taps, col 124 = 1.0 (bias row), rest 0.
    One contiguous 4MB DMA per step loads a whole-step mega-tile
    [128, 16bg x 8w x 128]; triple-buffered so the load never collides
    with the in-flight transposes (Tile serializes xbar transposes
    against other DMA completions).
  - per block: one xbar DMA transpose [128b, 1024] -> [128 taps, 8, 128b]
    builds the matmul lhsT (x-taps + h-taps + bias in one 128-row tile,
    single FWL LDWEIGHTS per window).
  - 8 matmuls (one per 8-wide l-window) vs banded weights [128, 160].
  - ScalarE: one sigmoid over i,f,o (960), tanh over g (320), and a
    deferred tanh(c) for the previous block (320, reordered to
    channel-major h layout with zero-guard halo columns).
  - VectorE: v=i*g, fc=f*c, c=fc+v (contiguous, 2x mode) and
    h = o*tanh(c) into the guarded channel-major h tile.
  - GpSimd: window-expands h (12 taps incl halos, via the guard cols)
    into next step's mega-tile h-region.
"""

import numpy as np

import concourse.bass as bass
import concourse.bacc as bacc
import concourse.tile as tile
import concourse.mybir as mybir
from concourse import bass_utils

dt = mybir.dt
ALU = mybir.AluOpType
ACT = mybir.ActivationFunctionType

TIME = 16
BATCH = 16384
C = 5
L = 64
NCORES = 8
BL = BATCH // NCORES          # 2048 per core
NW = 8                        # l-windows per row (l_seg = 8)
WJ = 12                       # taps per (window, channel): 8 + 4 halo
X_OFF = 0
H_OFF = 64
BIAS_COL = 124
CG = 68                       # channel block in clmaj layout: 2+64+2 guards


def make_wband(w_ih, w_hh, b_ih, b_hh):
    """Banded weight matrix [128, 160] fp16.

    Rows match window-block columns (after transpose these are lhsT
    partitions). Cols: G*40 + ch*8 + lam, G in (i,f,o,g) order.
    """
    refbase = (0, 5, 15, 10)  # i, f, o, g -> reference channel offsets
    wb = np.zeros((128, 160), np.float32)
    for row0, wmat in ((X_OFF, w_ih), (H_OFF, w_hh)):
        for c in range(C):
            for j in range(WJ):
                r = row0 + c * WJ + j
                for G in range(4):
                    scale = 2.0 if G == 3 else 1.0
                    for ch in range(C):
                        for lam in range(NW):
                            k = j - lam
                            if 0 <= k < 5:
                                wb[r, G * 40 + ch * 8 + lam] = (
                                    scale * wmat[refbase[G] + ch, c, k]
                                )
    bias = (np.asarray(b_ih) + np.asarray(b_hh)).astype(np.float32)
    for G in range(4):
        scale = 2.0 if G == 3 else 1.0
        for ch in range(C):
            for lam in range(NW):
                wb[BIAS_COL, G * 40 + ch * 8 + lam] = scale * bias[refbase[G] + ch]
    return wb.astype(np.float16)


def _ap(base, off, dims):
    """Manual AP over the same tensor as `base` (an AP), keeping its
    partition dim, with free dims `dims` at extra element offset `off`."""
    return bass.AP(
        tensor=base.tensor,
        offset=base.offset + off,
        ap=[list(base.ap[0])] + [list(d) for d in dims],
    )


def build_body(tc, out_dram, xs, wband_d, fcw5_d, consts_d, T, nbg):
    nc = tc.nc
    f16, f32 = dt.float16, dt.float32

    from contextlib import ExitStack
    es = ExitStack()
    pers = es.enter_context(tc.tile_pool(name="pers", bufs=1))
    psum_pool = es.enter_context(tc.tile_pool(name="psum", bufs=2, space="PSUM"))
    xht_pool = es.enter_context(tc.tile_pool(name="xht", bufs=3))
    ifog_pool = es.enter_context(tc.tile_pool(name="ifog", bufs=3))
    vfc_pool = es.enter_context(tc.tile_pool(name="vfc", bufs=2))
    tc_pool = es.enter_context(tc.tile_pool(name="tc", bufs=3))
    fin_pool = es.enter_context(tc.tile_pool(name="fin", bufs=2))

    wband = pers.tile([128, 160], f16, tag="wband")
    nc.sync.dma_start(out=wband, in_=wband_d)
    fcw5 = pers.tile([128, C * L], f16, tag="fcw5")
    nc.gpsimd.dma_start(
        out=fcw5,
        in_=bass.AP(tensor=fcw5_d.tensor, offset=fcw5_d.offset,
                    ap=[[0, 128], [1, C * L]]),
    )
    consts = pers.tile([128, 2], f32, tag="consts")
    nc.gpsimd.dma_start(
        out=consts,
        in_=bass.AP(tensor=consts_d.tensor, offset=consts_d.offset,
                    ap=[[0, 128], [1, 2]]),
    )
    fcbneg = consts[:, 0:1]
    negq = consts[:, 1:2]

    # step mega-tiles: [128, nbg * NW * 128] fp16, quad-buffered over t%4
    # (the h-tail TTs write into xh[(t+1)%4], whose zero-filled load must
    # complete a sweep earlier so the DVE never waits on it)
    NXB = 4
    colsz = nbg * NW * 128
    xh = [pers.tile([128, colsz], f16, tag=f"xh{k}", name=f"xh{k}")
          for k in range(NXB)]

    # c state: ping-pong per block pair
    npair = nbg // 2
    cbuf = [[pers.tile([128, 640], f16, tag=f"c{pp}_{pr}", name=f"c{pp}_{pr}")
             for pr in range(npair)] for pp in range(2)]
    for pr in range(npair):
        nc.vector.memset(cbuf[0][pr], 0.0)

    out_acc = pers.tile([128, nbg], f32, tag="out_acc")

    def load_step(t):
        src = xs[t].rearrange("(g p) c -> p g c", p=128)
        nc.gpsimd.dma_start(out=xh[t % NXB][:], in_=src)

    # prologue: steps 0..2
    load_step(0)
    load_step(1)
    load_step(2)

    ifog_tiles = {}
    pending = None

    def tail(pr, t):
        """Deferred pair tail: tanh(c), then h = o*tanh(c) window-expanded
        straight into the next step's mega-tile h-region (3 TTs: 8 core
        taps + left/right 2-tap halos from the neighbor windows; edge taps
        stay zero from the host-zeroed DMA load)."""
        ifog_f = ifog_tiles.pop(pr)[:]
        cn = cbuf[(t + 1) % 2][pr]
        tch = tc_pool.tile([128, 640], f16, tag="tch")
        nc.scalar.activation(out=tch, in_=cn[:], func=ACT.Tanh)
        base = pr * 2 * NW * 128 + H_OFF
        dst = xh[(t + 1) % NXB][:]
        # core 8 taps: pair dim merges into the window dim (strides align)
        nc.vector.tensor_tensor(
            out=_ap(dst, base + 2, [[128, 2 * NW], [WJ, C], [1, 8]]),
            in0=_ap(ifog_f, 80, [[160, 2 * NW], [8, C], [1, 8]]),
            in1=_ap(tch[:], 0, [[40, 2 * NW], [8, C], [1, 8]]),
            op=ALU.mult,
        )
        for half in range(2):
            hb = base + half * NW * 128
            io = half * NW * 160
            to = half * NW * 40
            nc.vector.tensor_tensor(
                out=_ap(dst, hb + 128, [[128, NW - 1], [WJ, C], [1, 2]]),
                in0=_ap(ifog_f, io + 80 + 6, [[160, NW - 1], [8, C], [1, 2]]),
                in1=_ap(tch[:], to + 6, [[40, NW - 1], [8, C], [1, 2]]),
                op=ALU.mult,
            )
            nc.vector.tensor_tensor(
                out=_ap(dst, hb + 10, [[128, NW - 1], [WJ, C], [1, 2]]),
                in0=_ap(ifog_f, io + 80 + 160, [[160, NW - 1], [8, C], [1, 2]]),
                in1=_ap(tch[:], to + 40, [[40, NW - 1], [8, C], [1, 2]]),
                op=ALU.mult,
            )

    for t in range(T):
        for pr in range(npair):
            xht = xht_pool.tile([128, 2 * NW, 128], f16, tag="xht")
            nc.sync.dma_start(
                out=xht[:],
                in_=xh[t % NXB][:, pr * 2 * NW * 128:(pr + 1) * 2 * NW * 128],
                transpose=True,
            )

            ifog = ifog_pool.tile([128, 2 * NW * 160], f16, tag="ifog")
            if pending is not None:
                tail(*pending)
            pending = (pr, t)

            for half in range(2):
                slot = psum_pool.tile([128, 4 * 512], f32, tag="gates")
                for w in range(NW):
                    col = (w // 2) * 512 + (w % 2) * 160
                    nc.tensor.matmul(
                        slot[:, col:col + 160],
                        lhsT=xht[:, half * NW + w, :],
                        rhs=wband[:],
                        start=True,
                        stop=True,
                    )
                nc.scalar.activation(
                    out=_ap(ifog[:], half * 1280,
                            [[320, 4], [160, 2], [1, 160]]),
                    in_=_ap(slot[:], 0, [[512, 4], [160, 2], [1, 160]]),
                    func=ACT.Sigmoid,
                )

            ifog_f = ifog[:]
            sl_i = _ap(ifog_f, 0, [[160, 2 * NW], [1, 40]])
            sl_f = _ap(ifog_f, 40, [[160, 2 * NW], [1, 40]])
            sl_g = _ap(ifog_f, 120, [[160, 2 * NW], [1, 40]])

            v = vfc_pool.tile([128, 640], f16, tag="v")
            nc.vector.tensor_tensor(out=v, in0=sl_i, in1=sl_g, op=ALU.mult)
            u = vfc_pool.tile([128, 640], f16, tag="u")
            nc.vector.scalar_tensor_tensor(
                out=u, in0=v[:], scalar=2.0, in1=sl_i,
                op0=ALU.mult, op1=ALU.subtract,
            )
            fc = vfc_pool.tile([128, 640], f16, tag="fc")
            co = cbuf[t % 2][pr]
            nc.vector.tensor_tensor(out=fc, in0=sl_f, in1=co[:], op=ALU.mult)
            cn = cbuf[(t + 1) % 2][pr]
            nc.vector.tensor_tensor(out=cn[:], in0=u[:], in1=fc[:], op=ALU.add)

            ifog_tiles[pr] = ifog

        if t + 3 < T:
            load_step(t + 3)

    tail(*pending)

    # --- final FC / combine ---
    hfin = xh[T % NXB][:]
    for bg in range(nbg):
        hview = _ap(hfin, bg * NW * 128 + H_OFF + 2, [[128, NW], [WJ, C], [1, 8]])
        fview = _ap(fcw5[:], 0, [[8, NW], [L, C], [1, 8]])
        tmp5 = fin_pool.tile([128, C * L], f32, tag="tmp5")
        nc.vector.tensor_tensor(
            out=_ap(tmp5[:], 0, [[8, NW], [L, C], [1, 8]]),
            in0=hview, in1=fview, op=ALU.mult)
        nraw = fin_pool.tile([128, C], f32, tag="nraw")
        nc.vector.tensor_reduce(
            out=nraw,
            in_=tmp5[:].rearrange("p (c l) -> p c l", l=L),
            axis=mybir.AxisListType.X,
            op=ALU.add,
        )
        pbar = fin_pool.tile([128, C], f32, tag="pbar")
        nc.scalar.activation(
            out=pbar, in_=nraw[:], func=ACT.Sigmoid, bias=fcbneg, scale=1.0
        )
        q2 = fin_pool.tile([128, 2], f32, tag="q2")
        nc.vector.tensor_tensor(out=q2, in0=pbar[:, 0:2], in1=pbar[:, 2:4],
                                op=ALU.mult)
        prod = fin_pool.tile([128, 1], f32, tag="prod")
        nc.vector.tensor_tensor(out=prod, in0=q2[:, 0:1], in1=q2[:, 1:2],
                                op=ALU.mult)
        nc.vector.tensor_tensor(out=prod, in0=prod[:], in1=pbar[:, 4:5],
                                op=ALU.mult)
        nc.scalar.activation(
            out=out_acc[:, bg:bg + 1], in_=prod[:], func=ACT.Identity,
            bias=1.0, scale=negq
        )
    nc.gpsimd.dma_start(out=out_dram, in_=out_acc[:])
    es.close()


def window_x(x):
    """[T, B, 5, 64] fp32 -> [T, B, NW*128] fp16 padded window blocks.

    Block w cols: 0..59 = x[b, c, w*8 + j - 2] (c*12 + j), col 124 = 1.0,
    everything else 0.
    """
    from numpy.lib.stride_tricks import sliding_window_view
    T, B = x.shape[0], x.shape[1]
    xp = np.pad(x, ((0, 0), (0, 0), (0, 0), (2, 2)))
    win = sliding_window_view(xp, WJ, axis=3)[:, :, :, ::NW, :]  # T,B,C,NW,WJ
    out = np.zeros((T, B, NW, 128), np.float16)
    out[:, :, :, : C * WJ] = (
        win.transpose(0, 1, 3, 2, 4).reshape(T, B, NW, C * WJ)
    )
    out[:, :, :, BIAS_COL] = 1.0
    return out.reshape(T, B, NW * 128)


def host_prep(w_ih, w_hh, b_ih, b_hh, fc_w, fc_b, baseline):
    wband = make_wband(np.asarray(w_ih), np.asarray(w_hh),
                       np.asarray(b_ih), np.asarray(b_hh))
    fcw = np.asarray(fc_w)[0].astype(np.float32)          # (64,)
    fcw5 = np.tile(-fcw, C)[None, :].astype(np.float16)    # (1, 320)
    base = float(np.asarray(baseline)[0])
    sig_base = 1.0 / (1.0 + np.exp(-base))
    consts = np.array([[-float(np.asarray(fc_b)[0]), -(1.0 - sig_base)]],
                      np.float32)
    return wband, fcw5, consts


def build_program(T, nbg):
    nc = bacc.Bacc("TRN2", target_bir_lowering=False, debug=False, num_devices=1)
    xs = nc.dram_tensor("xs", [T, nbg * 128, NW * 128], dt.float16,
                        kind="ExternalInput").ap()
    wband_d = nc.dram_tensor("wband", [128, 160], dt.float16,
                             kind="ExternalInput").ap()
    fcw5_d = nc.dram_tensor("fcw5", [1, C * L], dt.float16,
                            kind="ExternalInput").ap()
    consts_d = nc.dram_tensor("consts", [1, 2], dt.float32,
                              kind="ExternalInput").ap()
    out_d = nc.dram_tensor("out", [128, nbg], dt.float32,
                           kind="ExternalOutput").ap()
    with tile.TileContext(nc) as tc:
        build_body(tc, out_d, xs, wband_d, fcw5_d, consts_d, T, nbg)
    nc.compile()
    return nc


_PROG_CACHE = {}


def kernel(x, w_ih, w_hh, b_ih, b_hh, fc_w, fc_b, baseline):
    x = np.asarray(x)
    T, B = x.shape[0], x.shape[1]
    nbg = (B // NCORES) // 128
    key = (T, nbg)
    if key not in _PROG_CACHE:
        _PROG_CACHE[key] = build_program(T, nbg)
    nc = _PROG_CACHE[key]

    wband, fcw5, consts = host_prep(w_ih, w_hh, b_ih, b_hh, fc_w, fc_b, baseline)
    xw = window_x(x)
    bl = B // NCORES
    in_maps = []
    for core in range(NCORES):
        in_maps.append({
            "xs": np.ascontiguousarray(xw[:, core * bl: (core + 1) * bl]),
            "wband": wband,
            "fcw5": fcw5,
            "consts": consts,
        })
    res = bass_utils.run_bass_kernel_spmd(nc, in_maps, core_ids=list(range(NCORES)))
    out = np.concatenate([r["out"].T.reshape(-1) for r in res.results])
    return out.astype(np.float32)
